# revision 16
# baseline (speedup 1.0000x reference)
"""Trainium2 Bass kernel for CantorMultiheadFusion (sliding-window multi-head
attention, window K=64, H=16 heads, D=64, DIM=1024, x [2, 2048, 1024]).

Sharding: pure data-parallel over (batch, seq-quarter) -> 8 cores, 512 queries
each. Each core gets a 575-column key window of x^T whose out-of-range columns
are clamp-replicated on the host (exactly reproducing the reference's index
clamping), making the on-device program uniform across cores: plain banded
attention with a constant band mask, no collectives.

All matmuls run bf16 with K=128 contraction (per-head K tiles are zero-padded
to 128 partitions so the other head's Q rows are annihilated). Scores for a
pair of heads share one full PSUM bank so exp and window masking run as one
[128,512] op per pair; masking is a multiplicative bf16 0/1 mask after exp on
GpSimd; the softmax denominator comes from a ones-column appended to V. The
scores<->AV streams are software-pipelined so the PE never waits on the ACT
exp chain.
"""

import sys

for _p in ("/opt/trn_rl_repo", "/root/.axon_site/_ro/trn_rl_repo"):
    if _p not in sys.path:
        sys.path.append(_p)

import numpy as np
import ml_dtypes

import concourse.bass as bass
import concourse.tile as tile
from concourse import bacc, mybir
from concourse.bass_utils import run_bass_kernel_spmd
from concourse.masks import make_identity

BF16 = ml_dtypes.bfloat16

B, S, DIM = 2, 2048, 1024
H, D, K = 16, 64, 64
HALF = K // 2            # 32
CH = 512                 # queries per core
W = CH + K - 1           # 575 key-window columns per core
WPAD = CH + 128          # kt2 padded width (chunk-B reads for the last qblock)
NB = DIM // 128          # 8 dim chunks
N_CORES = 8
QB = CH // 128           # 4 query blocks per core
SCALE = 1.0 / np.sqrt(D)

_CACHED = {}


def _build_nc():
    fp32 = mybir.dt.float32
    bf16 = mybir.dt.bfloat16
    Exp = mybir.ActivationFunctionType.Exp

    nc = bacc.Bacc("TRN2", target_bir_lowering=False, debug=False,
                   num_devices=N_CORES)

    xw_d = nc.dram_tensor("xw", [DIM, W], bf16, kind="ExternalInput")
    wq_d = nc.dram_tensor("wq", [DIM, DIM], bf16, kind="ExternalInput")
    wk_d = nc.dram_tensor("wk", [DIM, DIM], bf16, kind="ExternalInput")
    wv_d = nc.dram_tensor("wv", [DIM, DIM], bf16, kind="ExternalInput")
    wo_d = nc.dram_tensor("wo", [DIM, DIM], bf16, kind="ExternalInput")
    bo_d = nc.dram_tensor("bo", [1, DIM], bf16, kind="ExternalInput")
    band_d = nc.dram_tensor("band", [128, 512], bf16, kind="ExternalInput")
    band01_d = nc.dram_tensor("band01", [128, 512], bf16, kind="ExternalInput")
    out_d = nc.dram_tensor("out", [CH, DIM], fp32, kind="ExternalOutput")

    with tile.TileContext(nc) as tc:
        with (
            tc.tile_pool(name="persist", bufs=1) as pp,
            tc.tile_pool(name="rot", bufs=3) as rot,
            tc.tile_pool(name="rot2", bufs=2) as rot2,
            tc.tile_pool(name="psum", bufs=2, space="PSUM") as ps,
        ):
            # ---- persistent SBUF tiles ----
            xw = [pp.tile([128, W], bf16, tag=f"xw{i}", name=f"xw{i}") for i in range(NB)]
            wq = [pp.tile([128, DIM], bf16, tag=f"wq{i}", name=f"wq{i}") for i in range(NB)]
            wk = [pp.tile([128, DIM], bf16, tag=f"wk{i}", name=f"wk{i}") for i in range(NB)]
            wv = [pp.tile([128, DIM], bf16, tag=f"wv{i}", name=f"wv{i}") for i in range(NB)]
            wo = [pp.tile([128, DIM], bf16, tag=f"wo{i}", name=f"wo{i}") for i in range(NB)]
            qt = [pp.tile([128, CH], bf16, tag=f"qt{i}", name=f"qt{i}") for i in range(NB)]
            # per-head K, feature rows zero-padded to 128, key cols zero-padded
            # to WPAD so every score matmul is a full [128,128] lhsT
            kt2 = [pp.tile([128, WPAD], bf16, tag=f"kt{i}", name=f"kt{i}") for i in range(H)]
            # V: tokens on partitions; per head 64 value cols + 1 ones col
            vt = [pp.tile([128, H, D + 1], bf16, tag=f"vt{i}", name=f"vt{i}") for i in range(5)]
            band = pp.tile([128, 512], bf16, tag="band")
            band01 = pp.tile([128, 512], bf16, tag="band01")
            bo_sb = pp.tile([1, DIM], bf16, tag="bo")
            ones = pp.tile([1, 128], bf16, tag="ones")
            ident = pp.tile([128, 128], bf16, tag="ident")

            # ---- input DMAs ----
            # First (wq, xw) chunk pair split 4-ways across issuing engines so
            # the first Q-proj matmul can start within ~3us.
            nc.sync.dma_start(out=wq[0][:, 0:256], in_=wq_d[0:128, 0:256])
            nc.gpsimd.dma_start(out=xw[0][:, 0:144], in_=xw_d[0:128, 0:144])
            nc.scalar.dma_start(out=xw[0][:, 288:432], in_=xw_d[0:128, 288:432])
            nc.sync.dma_start(out=wq[0][:, 256:512], in_=wq_d[0:128, 256:512])
            nc.gpsimd.dma_start(out=xw[0][:, 144:288], in_=xw_d[0:128, 144:288])
            nc.scalar.dma_start(out=xw[0][:, 432:W], in_=xw_d[0:128, 432:W])
            nc.sync.dma_start(out=wq[0][:, 512:DIM], in_=wq_d[0:128, 512:DIM])
            for i in range(1, NB):
                nc.sync.dma_start(out=wq[i][:], in_=wq_d[i * 128:(i + 1) * 128, :])
                nc.sync.dma_start(out=xw[i][:], in_=xw_d[i * 128:(i + 1) * 128, :])
            for i in range(NB):
                nc.sync.dma_start(out=wk[i][:], in_=wk_d[i * 128:(i + 1) * 128, :])
            nc.gpsimd.dma_start(out=band[:], in_=band_d[:])
            nc.gpsimd.dma_start(out=band01[:], in_=band01_d[:])
            for i in range(NB):
                nc.sync.dma_start(out=wv[i][:], in_=wv_d[i * 128:(i + 1) * 128, :])
            nc.gpsimd.dma_start(out=bo_sb[:], in_=bo_d[:])
            for i in range(NB):
                nc.sync.dma_start(out=wo[i][:], in_=wo_d[i * 128:(i + 1) * 128, :])

            nc.vector.memset(ones[:], 1.0)
            make_identity(nc, ident[:])
            for h in range(H):
                nc.vector.memset(kt2[h][:], 0.0)
            for t in range(5):
                nc.gpsimd.memset(vt[t][:], 1.0)

            # ---- Q projection (1/sqrt(D) pre-folded into wq on the host) ----
            for e in range(NB):
                acc = ps.tile([128, CH], fp32, tag="proj", name="proj_ps", bufs=4)
                for d in range(NB):
                    nc.tensor.matmul(acc[:], wq[d][:, e * 128:(e + 1) * 128],
                                     xw[d][:, HALF:HALF + CH],
                                     start=(d == 0), stop=(d == NB - 1))
                nc.vector.tensor_copy(qt[e][:], acc[:])

            # ---- K projection into zero-padded per-head tiles ----
            for e in range(NB):
                for t0, tn in ((0, 288), (288, W - 288)):
                    acc = ps.tile([128, CH], fp32, tag="proj", name="proj_ps", bufs=4)
                    for d in range(NB):
                        nc.tensor.matmul(acc[:, 0:tn], wk[d][:, e * 128:(e + 1) * 128],
                                         xw[d][:, t0:t0 + tn],
                                         start=(d == 0), stop=(d == NB - 1))
                    nc.scalar.copy(kt2[2 * e][0:64, t0:t0 + tn], acc[0:64, 0:tn])
                    nc.scalar.copy(kt2[2 * e + 1][64:128, t0:t0 + tn],
                                   acc[64:128, 0:tn])

            # ---- V projection: vt[t][tok, h, 0:64]; col 64 stays 1.0 ----
            for t in range(5):
                npart = 128 if t < 4 else W - 512    # 63 in last chunk
                for eh in range(2):
                    acc = ps.tile([128, 8, D], fp32, tag="proj", name="proj_ps", bufs=4)
                    for d in range(NB):
                        nc.tensor.matmul(acc[0:npart], xw[d][:, t * 128:t * 128 + npart],
                                         wv[d][:, eh * 512:(eh + 1) * 512],
                                         start=(d == 0), stop=(d == NB - 1))
                    nc.vector.tensor_copy(vt[t][0:npart, eh * 8:(eh + 1) * 8, 0:D],
                                          acc[0:npart])

            # ---- attention + output projection, per 128-query block ----
            for qb in range(QB):
                q0 = qb * 128
                attnout = rot2.tile([128, DIM], bf16, tag="attnout", name="attnout")

                def scores(p):
                    """Scores+band-bias+exp for head pair (2p, 2p+1), one bank.

                    The -30000 window bias is accumulated into PSUM by a fifth
                    matmul (identity @ band), so exp underflows to exact zeros
                    out-of-band and no separate mask op is needed."""
                    sc = ps.tile([128, 512], fp32, tag="sc", name="sc_ps")
                    for hh in range(2):
                        h = 2 * p + hh
                        nc.tensor.matmul(sc[:, 256 * hh:256 * hh + 128],
                                         kt2[h][:, q0:q0 + 128],
                                         qt[p][:, q0:q0 + 128],
                                         start=(hh == 0), stop=False)
                        nc.tensor.matmul(sc[:, 256 * hh + 128:256 * hh + 256],
                                         kt2[h][:, q0 + 128:q0 + 256],
                                         qt[p][:, q0:q0 + 128],
                                         start=False, stop=False)
                    e_sb = rot.tile([128, 512], bf16, tag="e", name="e_sb", bufs=4)
                    if p % 2 == 0:
                        nc.tensor.matmul(sc[:], ident[:], band[:],
                                         start=False, stop=True)
                        nc.scalar.activation(e_sb[:], sc[:], Exp)
                    else:
                        nc.tensor.matmul(sc[0:1, 0:1], ident[0:1, 0:1],
                                         band[0:1, 0:1], start=False, stop=True)
                        eraw = rot.tile([128, 512], bf16, tag="eraw",
                                        name="eraw", bufs=3)
                        nc.scalar.activation(eraw[:], sc[:], Exp)
                        nc.gpsimd.tensor_tensor(e_sb[:], eraw[:], band01[:],
                                                mybir.AluOpType.mult)
                    return e_sb

                def av_pair(p, e_sb):
                    for hh in range(2):
                        h = 2 * p + hh
                        av = ps.tile([128, D + 1], fp32, tag="av", name="av_ps")
                        nc.tensor.matmul(av[:], e_sb[:, 256 * hh:256 * hh + 128],
                                         vt[qb][:, h, :], start=True, stop=False)
                        nc.tensor.matmul(av[:], e_sb[:, 256 * hh + 128:256 * hh + 256],
                                         vt[qb + 1][:, h, :], start=False, stop=True)
                        invden = rot.tile([128, 1], fp32, tag="invden", name="invden")
                        nc.vector.reciprocal(invden[:], av[:, D:D + 1])
                        nc.vector.tensor_scalar_mul(attnout[:, h * D:(h + 1) * D],
                                                    av[:, 0:D], invden[:])

                es = [scores(0), scores(1)]
                for p in range(8):
                    if p + 2 < 8:
                        es.append(scores(p + 2))
                    av_pair(p, es[p])

                # transpose attnout to [dim, q] chunks for O projection
                attnT = []
                for c in range(NB):
                    trp = ps.tile([128, 128], bf16, tag="sc", name="tr_ps")
                    nc.tensor.transpose(trp[:], attnout[:, c * 128:(c + 1) * 128],
                                        ident[:])
                    at = rot2.tile([128, 128], bf16, tag=f"attnT{c}", name=f"attnT{c}")
                    nc.vector.tensor_copy(at[:], trp[:])
                    attnT.append(at)

                # O projection with bias folded in as a K=1 matmul
                out_sb = rot2.tile([128, DIM], fp32, tag="out", name="out_sb")
                for eh in range(2):
                    acc = ps.tile([128, 512], fp32, tag="proj", name="proj_ps", bufs=4)
                    for c in range(NB):
                        nc.tensor.matmul(acc[:], attnT[c][:],
                                         wo[c][:, eh * 512:(eh + 1) * 512],
                                         start=(c == 0), stop=False)
                    nc.tensor.matmul(acc[:], ones[:, 0:128],
                                     bo_sb[:, eh * 512:(eh + 1) * 512],
                                     start=False, stop=True)
                    nc.vector.tensor_copy(out_sb[:, eh * 512:(eh + 1) * 512], acc[:])
                    for j, de in enumerate((nc.sync, nc.gpsimd, nc.scalar,
                                            nc.sync)):
                        de.dma_start(
                            out=out_d[q0 + 32 * j:q0 + 32 * (j + 1),
                                      eh * 512:(eh + 1) * 512],
                            in_=out_sb[32 * j:32 * (j + 1),
                                       eh * 512:(eh + 1) * 512])

    nc.compile()
    return nc


def _host_prep(x, Wq, Wk, Wv, Wo, bo):
    """Per-core input maps: transposed bf16 weights + clamp-gathered x^T windows."""
    wqT = np.ascontiguousarray(Wq.T * SCALE).astype(BF16)   # fold 1/sqrt(D)
    wkT = np.ascontiguousarray(Wk.T).astype(BF16)
    wvT = np.ascontiguousarray(Wv.T).astype(BF16)
    woT = np.ascontiguousarray(Wo.T).astype(BF16)
    bo2 = bo.reshape(1, DIM).astype(BF16)

    # additive band bias, [key, query] layout, repeated for a head pair:
    # cols [A | B | A | B]; 0 in-band, -30000 out-of-band (exp underflows to 0)
    r = np.arange(128)[:, None]
    qq = np.arange(128)[None, :]
    bandA = np.where((r - qq >= 0) & (r - qq <= 63), 0.0, -30000.0)
    bandB = np.where((128 + r - qq >= 0) & (128 + r - qq <= 63), 0.0, -30000.0)
    band = np.concatenate([bandA, bandB, bandA, bandB], axis=1).astype(BF16)
    band01 = (np.concatenate([bandA, bandB, bandA, bandB], axis=1) == 0.0).astype(BF16)

    in_maps = []
    for core in range(N_CORES):
        b, c = divmod(core, QB)
        c0 = c * CH
        idx = np.clip(np.arange(c0 - HALF, c0 + CH + HALF - 1), 0, S - 1)
        xw = np.ascontiguousarray(x[b].T[:, idx]).astype(BF16)
        in_maps.append({
            "xw": xw, "wq": wqT, "wk": wkT, "wv": wvT, "wo": woT,
            "bo": bo2, "band": band, "band01": band01,
        })
    return in_maps


def _run(x, Wq, Wk, Wv, Wo, bo, trace=False, **kw):
    if "nc" not in _CACHED:
        _CACHED["nc"] = _build_nc()
    nc = _CACHED["nc"]
    in_maps = _host_prep(x, Wq, Wk, Wv, Wo, bo)
    res = run_bass_kernel_spmd(nc, in_maps, list(range(N_CORES)),
                               trace=trace, **kw)
    out = np.empty((B, S, DIM), np.float32)
    for core in range(N_CORES):
        b, c = divmod(core, QB)
        out[b, c * CH:(c + 1) * CH] = res.results[core]["out"]
    return out, res


def kernel(x, cantor_coords, Wq, Wk, Wv, Wo, bo):
    x = np.asarray(x, dtype=np.float32)
    out, _ = _run(x, np.asarray(Wq), np.asarray(Wk), np.asarray(Wv),
                  np.asarray(Wo), np.asarray(bo))
    return out


# revision 17
# speedup vs baseline: 1.0817x; 1.0817x over previous
"""Trainium2 Bass kernel for CantorMultiheadFusion (sliding-window multi-head
attention, window K=64, H=16 heads, D=64, DIM=1024, x [2, 2048, 1024]).

Sharding: pure data-parallel over (batch, seq-quarter) -> 8 cores, 512 queries
each. Each core gets a 575-column key window of x^T whose out-of-range columns
are clamp-replicated on the host (exactly reproducing the reference's index
clamping), making the on-device program uniform across cores: plain banded
attention with a constant band mask, no collectives.

All matmuls run bf16 with K=128 contraction (per-head K tiles are zero-padded
to 128 partitions so the other head's Q rows are annihilated). Scores for a
pair of heads share one full PSUM bank so exp and window masking run as one
[128,512] op per pair; masking is a multiplicative bf16 0/1 mask after exp on
GpSimd; the softmax denominator comes from a ones-column appended to V. The
scores<->AV streams are software-pipelined so the PE never waits on the ACT
exp chain.
"""

import sys

for _p in ("/opt/trn_rl_repo", "/root/.axon_site/_ro/trn_rl_repo"):
    if _p not in sys.path:
        sys.path.append(_p)

import numpy as np
import ml_dtypes

import concourse.bass as bass
import concourse.tile as tile
from concourse import bacc, mybir
from concourse.bass_utils import run_bass_kernel_spmd
from concourse.masks import make_identity

BF16 = ml_dtypes.bfloat16

B, S, DIM = 2, 2048, 1024
H, D, K = 16, 64, 64
HALF = K // 2            # 32
CH = 512                 # queries per core
W = CH + K - 1           # 575 key-window columns per core
WPAD = CH + 128          # kt2 padded width (chunk-B reads for the last qblock)
NB = DIM // 128          # 8 dim chunks
N_CORES = 8
QB = CH // 128           # 4 query blocks per core
SCALE = 1.0 / np.sqrt(D)

_CACHED = {}


def _build_nc():
    fp32 = mybir.dt.float32
    bf16 = mybir.dt.bfloat16
    Exp = mybir.ActivationFunctionType.Exp

    nc = bacc.Bacc("TRN2", target_bir_lowering=False, debug=False,
                   num_devices=N_CORES)

    xw_d = nc.dram_tensor("xw", [DIM, W], bf16, kind="ExternalInput")
    wq_d = nc.dram_tensor("wq", [DIM, DIM], bf16, kind="ExternalInput")
    wk_d = nc.dram_tensor("wk", [DIM, DIM], bf16, kind="ExternalInput")
    wv_d = nc.dram_tensor("wv", [DIM, DIM], bf16, kind="ExternalInput")
    wo_d = nc.dram_tensor("wo", [DIM, DIM], bf16, kind="ExternalInput")
    bo_d = nc.dram_tensor("bo", [1, DIM], bf16, kind="ExternalInput")
    band_d = nc.dram_tensor("band", [128, 512], bf16, kind="ExternalInput")
    out_d = nc.dram_tensor("out", [CH, DIM], fp32, kind="ExternalOutput")

    with tile.TileContext(nc) as tc:
        with (
            tc.tile_pool(name="persist", bufs=1) as pp,
            tc.tile_pool(name="rot", bufs=3) as rot,
            tc.tile_pool(name="rot2", bufs=2) as rot2,
            tc.tile_pool(name="psum", bufs=2, space="PSUM") as ps,
        ):
            # ---- persistent SBUF tiles ----
            xw = [pp.tile([128, W], bf16, tag=f"xw{i}", name=f"xw{i}") for i in range(NB)]
            wq = [pp.tile([128, DIM], bf16, tag=f"wq{i}", name=f"wq{i}") for i in range(NB)]
            wk = [pp.tile([128, DIM], bf16, tag=f"wk{i}", name=f"wk{i}") for i in range(NB)]
            wv = [pp.tile([128, DIM], bf16, tag=f"wv{i}", name=f"wv{i}") for i in range(NB)]
            wo = [pp.tile([128, DIM], bf16, tag=f"wo{i}", name=f"wo{i}") for i in range(NB)]
            qt = [pp.tile([128, CH], bf16, tag=f"qt{i}", name=f"qt{i}") for i in range(NB)]
            # per-head K, feature rows zero-padded to 128, key cols zero-padded
            # to WPAD so every score matmul is a full [128,128] lhsT
            kt2 = [pp.tile([128, WPAD], bf16, tag=f"kt{i}", name=f"kt{i}") for i in range(H)]
            # V: tokens on partitions; per head 64 value cols + 1 ones col
            vt = [pp.tile([128, H, D + 1], bf16, tag=f"vt{i}", name=f"vt{i}") for i in range(5)]
            band = pp.tile([128, 512], bf16, tag="band")
            bo_sb = pp.tile([1, DIM], bf16, tag="bo")
            ones = pp.tile([1, 128], bf16, tag="ones")
            ident = pp.tile([128, 128], bf16, tag="ident")

            # ---- input DMAs ----
            # First (wq, xw) chunk pair split 4-ways across issuing engines so
            # the first Q-proj matmul can start within ~3us.
            nc.sync.dma_start(out=wq[0][:, 0:256], in_=wq_d[0:128, 0:256])
            nc.gpsimd.dma_start(out=xw[0][:, 0:144], in_=xw_d[0:128, 0:144])
            nc.scalar.dma_start(out=xw[0][:, 288:432], in_=xw_d[0:128, 288:432])
            nc.sync.dma_start(out=wq[0][:, 256:512], in_=wq_d[0:128, 256:512])
            nc.gpsimd.dma_start(out=xw[0][:, 144:288], in_=xw_d[0:128, 144:288])
            nc.scalar.dma_start(out=xw[0][:, 432:W], in_=xw_d[0:128, 432:W])
            nc.sync.dma_start(out=wq[0][:, 512:DIM], in_=wq_d[0:128, 512:DIM])
            for i in range(1, NB):
                nc.sync.dma_start(out=wq[i][:], in_=wq_d[i * 128:(i + 1) * 128, :])
                nc.sync.dma_start(out=xw[i][:], in_=xw_d[i * 128:(i + 1) * 128, :])
            for i in range(NB):
                nc.sync.dma_start(out=wk[i][:], in_=wk_d[i * 128:(i + 1) * 128, :])
            nc.gpsimd.dma_start(out=band[:], in_=band_d[:])
            for i in range(NB):
                nc.sync.dma_start(out=wv[i][:], in_=wv_d[i * 128:(i + 1) * 128, :])
            nc.gpsimd.dma_start(out=bo_sb[:], in_=bo_d[:])
            for i in range(NB):
                nc.sync.dma_start(out=wo[i][:], in_=wo_d[i * 128:(i + 1) * 128, :])

            nc.vector.memset(ones[:], 1.0)
            make_identity(nc, ident[:])
            for h in range(H):
                nc.vector.memset(kt2[h][:], 0.0)
            for t in range(5):
                nc.gpsimd.memset(vt[t][:], 1.0)

            # ---- Q projection (1/sqrt(D) pre-folded into wq on the host) ----
            for e in range(NB):
                acc = ps.tile([128, CH], fp32, tag="proj", name="proj_ps", bufs=4)
                for d in range(NB):
                    nc.tensor.matmul(acc[:], wq[d][:, e * 128:(e + 1) * 128],
                                     xw[d][:, HALF:HALF + CH],
                                     start=(d == 0), stop=(d == NB - 1))
                nc.vector.tensor_copy(qt[e][:], acc[:])

            # ---- K projection into zero-padded per-head tiles ----
            for e in range(NB):
                for t0, tn in ((0, 288), (288, W - 288)):
                    acc = ps.tile([128, CH], fp32, tag="proj", name="proj_ps", bufs=4)
                    for d in range(NB):
                        nc.tensor.matmul(acc[:, 0:tn], wk[d][:, e * 128:(e + 1) * 128],
                                         xw[d][:, t0:t0 + tn],
                                         start=(d == 0), stop=(d == NB - 1))
                    nc.scalar.copy(kt2[2 * e][0:64, t0:t0 + tn], acc[0:64, 0:tn])
                    nc.scalar.copy(kt2[2 * e + 1][64:128, t0:t0 + tn],
                                   acc[64:128, 0:tn])

            # ---- V projection: vt[t][tok, h, 0:64]; col 64 stays 1.0 ----
            for t in range(5):
                npart = 128 if t < 4 else W - 512    # 63 in last chunk
                for eh in range(2):
                    acc = ps.tile([128, 8, D], fp32, tag="proj", name="proj_ps", bufs=4)
                    for d in range(NB):
                        nc.tensor.matmul(acc[0:npart], xw[d][:, t * 128:t * 128 + npart],
                                         wv[d][:, eh * 512:(eh + 1) * 512],
                                         start=(d == 0), stop=(d == NB - 1))
                    nc.vector.tensor_copy(vt[t][0:npart, eh * 8:(eh + 1) * 8, 0:D],
                                          acc[0:npart])

            # ---- attention + output projection, per 128-query block ----
            for qb in range(QB):
                q0 = qb * 128
                attnout = rot2.tile([128, DIM], bf16, tag="attnout", name="attnout")

                def scores(p):
                    """Scores+band-bias+exp for head pair (2p, 2p+1), one bank.

                    The -30000 window bias is accumulated into PSUM by a fifth
                    matmul (identity @ band), so exp underflows to exact zeros
                    out-of-band and no separate mask op is needed."""
                    sc = ps.tile([128, 512], fp32, tag="sc", name="sc_ps")
                    for hh in range(2):
                        h = 2 * p + hh
                        nc.tensor.matmul(sc[:, 256 * hh:256 * hh + 128],
                                         kt2[h][:, q0:q0 + 128],
                                         qt[p][:, q0:q0 + 128],
                                         start=(hh == 0), stop=False)
                        nc.tensor.matmul(sc[:, 256 * hh + 128:256 * hh + 256],
                                         kt2[h][:, q0 + 128:q0 + 256],
                                         qt[p][:, q0:q0 + 128],
                                         start=False, stop=False)
                    nc.tensor.matmul(sc[:], ident[:], band[:],
                                     start=False, stop=True)
                    e_sb = rot.tile([128, 512], bf16, tag="e", name="e_sb", bufs=4)
                    nc.scalar.activation(e_sb[:], sc[:], Exp)
                    return e_sb

                def av_pair(p, e_sb):
                    for hh in range(2):
                        h = 2 * p + hh
                        av = ps.tile([128, D + 1], fp32, tag="av", name="av_ps")
                        nc.tensor.matmul(av[:], e_sb[:, 256 * hh:256 * hh + 128],
                                         vt[qb][:, h, :], start=True, stop=False)
                        nc.tensor.matmul(av[:], e_sb[:, 256 * hh + 128:256 * hh + 256],
                                         vt[qb + 1][:, h, :], start=False, stop=True)
                        invden = rot.tile([128, 1], fp32, tag="invden", name="invden")
                        nc.vector.reciprocal(invden[:], av[:, D:D + 1])
                        nc.vector.tensor_scalar_mul(attnout[:, h * D:(h + 1) * D],
                                                    av[:, 0:D], invden[:])

                es = [scores(0), scores(1)]
                for p in range(8):
                    if p + 2 < 8:
                        es.append(scores(p + 2))
                    av_pair(p, es[p])

                # transpose attnout to [dim, q] chunks for O projection
                attnT = []
                for c in range(NB):
                    trp = ps.tile([128, 128], bf16, tag="sc", name="tr_ps")
                    nc.tensor.transpose(trp[:], attnout[:, c * 128:(c + 1) * 128],
                                        ident[:])
                    at = rot2.tile([128, 128], bf16, tag=f"attnT{c}", name=f"attnT{c}")
                    nc.vector.tensor_copy(at[:], trp[:])
                    attnT.append(at)

                # O projection with bias folded in as a K=1 matmul
                out_sb = rot2.tile([128, DIM], fp32, tag="out", name="out_sb")
                for eh in range(2):
                    acc = ps.tile([128, 512], fp32, tag="proj", name="proj_ps", bufs=4)
                    for c in range(NB):
                        nc.tensor.matmul(acc[:], attnT[c][:],
                                         wo[c][:, eh * 512:(eh + 1) * 512],
                                         start=(c == 0), stop=False)
                    nc.tensor.matmul(acc[:], ones[:, 0:128],
                                     bo_sb[:, eh * 512:(eh + 1) * 512],
                                     start=False, stop=True)
                    nc.vector.tensor_copy(out_sb[:, eh * 512:(eh + 1) * 512], acc[:])
                    for j, de in enumerate((nc.sync, nc.gpsimd, nc.scalar,
                                            nc.sync)):
                        de.dma_start(
                            out=out_d[q0 + 32 * j:q0 + 32 * (j + 1),
                                      eh * 512:(eh + 1) * 512],
                            in_=out_sb[32 * j:32 * (j + 1),
                                       eh * 512:(eh + 1) * 512])

    nc.compile()
    return nc


def _host_prep(x, Wq, Wk, Wv, Wo, bo):
    """Per-core input maps: transposed bf16 weights + clamp-gathered x^T windows."""
    wqT = np.ascontiguousarray(Wq.T * SCALE).astype(BF16)   # fold 1/sqrt(D)
    wkT = np.ascontiguousarray(Wk.T).astype(BF16)
    wvT = np.ascontiguousarray(Wv.T).astype(BF16)
    woT = np.ascontiguousarray(Wo.T).astype(BF16)
    bo2 = bo.reshape(1, DIM).astype(BF16)

    # additive band bias, [key, query] layout, repeated for a head pair:
    # cols [A | B | A | B]; 0 in-band, -30000 out-of-band (exp underflows to 0)
    r = np.arange(128)[:, None]
    qq = np.arange(128)[None, :]
    bandA = np.where((r - qq >= 0) & (r - qq <= 63), 0.0, -30000.0)
    bandB = np.where((128 + r - qq >= 0) & (128 + r - qq <= 63), 0.0, -30000.0)
    band = np.concatenate([bandA, bandB, bandA, bandB], axis=1).astype(BF16)

    in_maps = []
    for core in range(N_CORES):
        b, c = divmod(core, QB)
        c0 = c * CH
        idx = np.clip(np.arange(c0 - HALF, c0 + CH + HALF - 1), 0, S - 1)
        xw = np.ascontiguousarray(x[b].T[:, idx]).astype(BF16)
        in_maps.append({
            "xw": xw, "wq": wqT, "wk": wkT, "wv": wvT, "wo": woT,
            "bo": bo2, "band": band,
        })
    return in_maps


def _run(x, Wq, Wk, Wv, Wo, bo, trace=False, **kw):
    if "nc" not in _CACHED:
        _CACHED["nc"] = _build_nc()
    nc = _CACHED["nc"]
    in_maps = _host_prep(x, Wq, Wk, Wv, Wo, bo)
    res = run_bass_kernel_spmd(nc, in_maps, list(range(N_CORES)),
                               trace=trace, **kw)
    out = np.empty((B, S, DIM), np.float32)
    for core in range(N_CORES):
        b, c = divmod(core, QB)
        out[b, c * CH:(c + 1) * CH] = res.results[core]["out"]
    return out, res


def kernel(x, cantor_coords, Wq, Wk, Wv, Wo, bo):
    x = np.asarray(x, dtype=np.float32)
    out, _ = _run(x, np.asarray(Wq), np.asarray(Wk), np.asarray(Wv),
                  np.asarray(Wo), np.asarray(bo))
    return out
